# revision 12
# baseline (speedup 1.0000x reference)
"""CliffordLinear kernel for Trainium2 (8 NeuronCores, data parallel).

The reference applies 2016 sequential Givens rotations (one per (i,j) pair,
i<j, dim=64) to every row of x, then adds a bias. Each rotation is linear in
x, so the whole sequence composes into a single 64x64 matrix R with
out = x @ R + bias. R is computed on the host (float64, 2016 tiny updates);
the device does a streaming fp32 matmul.

Device layout: the tensor engine contracts over the partition axis, so x
must be partition-major in the feature dim. Each core's shard (65536, 64)
is pre-arranged on host into tiles of [128, TILE_COLS] where partition
p = b*64+d holds feature d of row-block b (two 32768-row blocks stacked).
The stationary weight is W = blockdiag(R, R) [128, 128] so one matmul
processes both blocks with all 128 partitions active. Tiles are stored
tile-major in DRAM ([T, 128, C]) so every DMA is a single fully contiguous
block. Output comes back in the same layout and is un-arranged on host.
"""

import numpy as np

DIM = 64
NROWS = 524288
NCORES = 8
SHARD = NROWS // NCORES  # 65536 rows per core
HALF = SHARD // 2        # 32768 columns per stacked block
TILE_COLS = 2048         # columns per DMA tile (128*2048*4 = 1 MiB)
MM_COLS = 512            # moving-operand columns per matmul (fp32 max)

_BASS_CACHE = {}


def _compose_rotation(coeffs64):
    """R such that applying the reference rotation sequence == x @ R."""
    ii, jj = np.triu_indices(DIM, k=1)
    c = np.cos(coeffs64)
    s = np.sin(coeffs64)
    R = np.eye(DIM, dtype=np.float64)
    for k in range(len(ii)):
        i, j = int(ii[k]), int(jj[k])
        ri = R[:, i].copy()
        rj = R[:, j].copy()
        R[:, i] = c[k] * ri - s[k] * rj
        R[:, j] = s[k] * ri + c[k] * rj
    return R


def _pack_shard(xs, tile_cols):
    """(SHARD, DIM) -> [T, 128, tile_cols] tile-major device layout."""
    t = HALF // tile_cols
    x2 = xs.reshape(2, HALF, DIM).transpose(0, 2, 1).reshape(128, HALF)
    return np.ascontiguousarray(
        x2.reshape(128, t, tile_cols).transpose(1, 0, 2)
    )


def _unpack_shard(o3, tile_cols):
    """[T, 128, tile_cols] -> (SHARD, DIM)."""
    o2 = np.asarray(o3).transpose(1, 0, 2).reshape(128, HALF)
    return o2.reshape(2, DIM, HALF).transpose(0, 2, 1).reshape(SHARD, DIM)


MM_DTYPE = "f32"  # flipped to "f32r" only if HW numerics check out


def _build_bass(half=HALF, tile_cols=TILE_COLS, n_cores=NCORES, reps=1,
                mm_dtype=None, mode="full", io_bufs=3):
    if mm_dtype is None:
        mm_dtype = MM_DTYPE
    import concourse.bass as bass
    import concourse.bacc as bacc
    import concourse.mybir as mybir
    import concourse.tile as tile

    f32 = mybir.dt.float32
    fmm = mybir.dt.float32r if mm_dtype == "f32r" else f32
    nc = bacc.Bacc(
        "TRN2", target_bir_lowering=False, debug=False, num_devices=n_cores
    )
    n_tiles = half // tile_cols
    mm_per_tile = tile_cols // MM_COLS

    x_d = nc.dram_tensor("x2", [n_tiles, 128, tile_cols], fmm,
                         kind="ExternalInput")
    w_d = nc.dram_tensor("w", [128, 128], fmm, kind="ExternalInput")
    b_d = nc.dram_tensor("b2", [128, 1], f32, kind="ExternalInput")
    o_d = nc.dram_tensor("o2", [n_tiles, 128, tile_cols], f32,
                         kind="ExternalOutput")

    with tile.TileContext(nc) as tc:
        with (
            tc.tile_pool(name="const", bufs=1) as cpool,
            tc.tile_pool(name="io", bufs=io_bufs) as iopool,
            tc.tile_pool(name="xp", bufs=1) as xpool,
            tc.tile_pool(name="ps", bufs=8, space=bass.MemorySpace.PSUM) as pspool,
        ):
            w = cpool.tile([128, 128], fmm)
            nc.sync.dma_start(w[:], w_d[:])
            bb = cpool.tile([128, 1], f32)
            nc.sync.dma_start(bb[:], b_d[:])
            for _rep in range(reps):
                if mode == "preload":
                    # all xin tiles resident at once: PE never waits mid-run
                    # and stays HAM-warm (dense back-to-back matmuls).
                    # bufs=1 per tag — 16 tags x 8KiB/part = 128KiB/part.
                    xins = []
                    for t in range(n_tiles):
                        xin = xpool.tile([128, tile_cols], fmm, tag=f"xin{t}")
                        nc.sync.dma_start(xin[:], x_d[t])
                        xins.append(xin)
                    for t in range(n_tiles):
                        out = iopool.tile([128, tile_cols], f32, tag="out")
                        for u in range(mm_per_tile):
                            ps = pspool.tile([128, MM_COLS], f32)
                            nc.tensor.matmul(
                                ps[:],
                                w[:],
                                xins[t][:, u * MM_COLS:(u + 1) * MM_COLS],
                                start=True,
                                stop=True,
                            )
                            nc.vector.tensor_scalar_add(
                                out[:, u * MM_COLS:(u + 1) * MM_COLS],
                                ps[:], bb[:],
                            )
                        nc.scalar.dma_start(o_d[t], out[:])
                    continue
                for t in range(n_tiles):
                    xin = iopool.tile([128, tile_cols], fmm, tag="xin")
                    nc.sync.dma_start(xin[:], x_d[t])
                    if mode == "dma_only":
                        nc.scalar.dma_start(o_d[t], xin[:])
                        continue
                    out = iopool.tile([128, tile_cols], f32, tag="out")
                    for u in range(mm_per_tile):
                        ps = pspool.tile([128, MM_COLS], f32)
                        nc.tensor.matmul(
                            ps[:],
                            w[:],
                            xin[:, u * MM_COLS:(u + 1) * MM_COLS],
                            start=True,
                            stop=True,
                        )
                        nc.vector.tensor_scalar_add(
                            out[:, u * MM_COLS:(u + 1) * MM_COLS], ps[:], bb[:]
                        )
                    # store on the ACT HWDGE ring so loads (SP ring) never
                    # head-of-line block behind a store's wait
                    nc.scalar.dma_start(o_d[t], out[:])
    nc.compile()
    return nc


def kernel(x, bivector_coeffs, bias):
    from concourse.bass_utils import run_bass_kernel_spmd

    x = np.ascontiguousarray(np.asarray(x, dtype=np.float32))
    coeffs = np.asarray(bivector_coeffs, dtype=np.float64)
    bias = np.asarray(bias, dtype=np.float32)

    R32 = _compose_rotation(coeffs).astype(np.float32)
    W = np.zeros((128, 128), dtype=np.float32)
    W[:DIM, :DIM] = R32
    W[DIM:, DIM:] = R32
    b2 = np.ascontiguousarray(np.tile(bias, 2).reshape(128, 1))

    key = (HALF, TILE_COLS, NCORES, 1)
    if key not in _BASS_CACHE:
        _BASS_CACHE[key] = _build_bass(
            half=HALF, tile_cols=TILE_COLS, n_cores=NCORES, reps=1,
            mm_dtype="f32", mode="preload",
        )
    nc = _BASS_CACHE[key]

    in_maps = []
    for r in range(NCORES):
        xs = x[r * SHARD:(r + 1) * SHARD]
        in_maps.append(
            {"x2": _pack_shard(xs, TILE_COLS), "w": W, "b2": b2}
        )

    res = run_bass_kernel_spmd(
        nc, in_maps, core_ids=list(range(NCORES)), trace=False
    )

    out = np.empty((NROWS, DIM), dtype=np.float32)
    for r in range(NCORES):
        out[r * SHARD:(r + 1) * SHARD] = _unpack_shard(
            res.results[r]["o2"], TILE_COLS
        )
    return out


# revision 15
# speedup vs baseline: 1.0027x; 1.0027x over previous
"""CliffordLinear kernel for Trainium2 (8 NeuronCores, data parallel).

The reference applies 2016 sequential Givens rotations (one per (i,j) pair,
i<j, dim=64) to every row of x, then adds a bias. Each rotation is linear in
x, so the whole sequence composes into a single 64x64 matrix R with
out = x @ R + bias. R is computed on the host (float64, 2016 tiny updates);
the device does a streaming fp32 matmul.

Device layout: the tensor engine contracts over the partition axis, so x
must be partition-major in the feature dim. Each core's shard (65536, 64)
is pre-arranged on host into tiles of [128, TILE_COLS] where partition
p = b*64+d holds feature d of row-block b (two 32768-row blocks stacked).
The stationary weight is W = blockdiag(R, R) [128, 128] so one matmul
processes both blocks with all 128 partitions active. Tiles are stored
tile-major in DRAM ([T, 128, C]) so every DMA is a single fully contiguous
block. Output comes back in the same layout and is un-arranged on host.
"""

import numpy as np

DIM = 64
NROWS = 524288
NCORES = 8
SHARD = NROWS // NCORES  # 65536 rows per core
HALF = SHARD // 2        # 32768 columns per stacked block
TILE_COLS = 2048         # columns per DMA tile (128*2048*4 = 1 MiB)
MM_COLS = 512            # moving-operand columns per matmul (fp32 max)

_BASS_CACHE = {}


def _compose_rotation(coeffs64):
    """R such that applying the reference rotation sequence == x @ R."""
    ii, jj = np.triu_indices(DIM, k=1)
    c = np.cos(coeffs64)
    s = np.sin(coeffs64)
    R = np.eye(DIM, dtype=np.float64)
    for k in range(len(ii)):
        i, j = int(ii[k]), int(jj[k])
        ri = R[:, i].copy()
        rj = R[:, j].copy()
        R[:, i] = c[k] * ri - s[k] * rj
        R[:, j] = s[k] * ri + c[k] * rj
    return R


def _pack_shard(xs, tile_cols):
    """(SHARD, DIM) -> [T, 128, tile_cols] tile-major device layout."""
    t = HALF // tile_cols
    x2 = xs.reshape(2, HALF, DIM).transpose(0, 2, 1).reshape(128, HALF)
    return np.ascontiguousarray(
        x2.reshape(128, t, tile_cols).transpose(1, 0, 2)
    )


def _unpack_shard(o3, tile_cols):
    """[T, 128, tile_cols] -> (SHARD, DIM)."""
    o2 = np.asarray(o3).transpose(1, 0, 2).reshape(128, HALF)
    return o2.reshape(2, DIM, HALF).transpose(0, 2, 1).reshape(SHARD, DIM)


MM_DTYPE = "f32"  # flipped to "f32r" only if HW numerics check out


def _build_bass(half=HALF, tile_cols=TILE_COLS, n_cores=NCORES, reps=1,
                mm_dtype=None, mode="full", io_bufs=3, ring_split=False):
    if mm_dtype is None:
        mm_dtype = MM_DTYPE
    import concourse.bass as bass
    import concourse.bacc as bacc
    import concourse.mybir as mybir
    import concourse.tile as tile

    f32 = mybir.dt.float32
    fmm = mybir.dt.float32r if mm_dtype == "f32r" else f32
    nc = bacc.Bacc(
        "TRN2", target_bir_lowering=False, debug=False, num_devices=n_cores
    )
    n_tiles = half // tile_cols
    mm_per_tile = tile_cols // MM_COLS

    x_d = nc.dram_tensor("x2", [n_tiles, 128, tile_cols], fmm,
                         kind="ExternalInput")
    w_d = nc.dram_tensor("w", [128, 128], fmm, kind="ExternalInput")
    b_d = nc.dram_tensor("b2", [128, 1], f32, kind="ExternalInput")
    o_d = nc.dram_tensor("o2", [n_tiles, 128, tile_cols], f32,
                         kind="ExternalOutput")

    with tile.TileContext(nc) as tc:
        with (
            tc.tile_pool(name="const", bufs=1) as cpool,
            tc.tile_pool(name="io", bufs=io_bufs) as iopool,
            tc.tile_pool(name="xp", bufs=1) as xpool,
            tc.tile_pool(name="ps", bufs=8, space=bass.MemorySpace.PSUM) as pspool,
        ):
            w = cpool.tile([128, 128], fmm)
            nc.sync.dma_start(w[:], w_d[:])
            bb = cpool.tile([128, 1], f32)
            nc.sync.dma_start(bb[:], b_d[:])
            for _rep in range(reps):
                if mode == "preload":
                    # all xin tiles resident at once: PE never waits mid-run
                    # and stays HAM-warm (dense back-to-back matmuls).
                    # bufs=1 per tag — 16 tags x 8KiB/part = 128KiB/part.
                    # ring_split: alternate loads across BOTH HWDGE rings
                    # (SP + ACT) so the preload burst isn't single-ring
                    # limited; stores take the opposite-parity ring.
                    xins = []
                    for t in range(n_tiles):
                        xin = xpool.tile([128, tile_cols], fmm, tag=f"xin{t}")
                        ld = nc.sync if (not ring_split or t % 2 == 0) \
                            else nc.scalar
                        ld.dma_start(xin[:], x_d[t])
                        xins.append(xin)
                    for t in range(n_tiles):
                        out = iopool.tile([128, tile_cols], f32, tag="out")
                        for u in range(mm_per_tile):
                            ps = pspool.tile([128, MM_COLS], f32)
                            nc.tensor.matmul(
                                ps[:],
                                w[:],
                                xins[t][:, u * MM_COLS:(u + 1) * MM_COLS],
                                start=True,
                                stop=True,
                            )
                            nc.vector.tensor_scalar_add(
                                out[:, u * MM_COLS:(u + 1) * MM_COLS],
                                ps[:], bb[:],
                            )
                        st = nc.scalar if (not ring_split or t % 2 == 0) \
                            else nc.sync
                        st.dma_start(o_d[t], out[:])
                    continue
                for t in range(n_tiles):
                    xin = iopool.tile([128, tile_cols], fmm, tag="xin")
                    nc.sync.dma_start(xin[:], x_d[t])
                    if mode == "dma_only":
                        nc.scalar.dma_start(o_d[t], xin[:])
                        continue
                    out = iopool.tile([128, tile_cols], f32, tag="out")
                    for u in range(mm_per_tile):
                        ps = pspool.tile([128, MM_COLS], f32)
                        nc.tensor.matmul(
                            ps[:],
                            w[:],
                            xin[:, u * MM_COLS:(u + 1) * MM_COLS],
                            start=True,
                            stop=True,
                        )
                        nc.vector.tensor_scalar_add(
                            out[:, u * MM_COLS:(u + 1) * MM_COLS], ps[:], bb[:]
                        )
                    # store on the ACT HWDGE ring so loads (SP ring) never
                    # head-of-line block behind a store's wait
                    nc.scalar.dma_start(o_d[t], out[:])
    nc.compile()
    return nc


def kernel(x, bivector_coeffs, bias):
    from concourse.bass_utils import run_bass_kernel_spmd

    x = np.ascontiguousarray(np.asarray(x, dtype=np.float32))
    coeffs = np.asarray(bivector_coeffs, dtype=np.float64)
    bias = np.asarray(bias, dtype=np.float32)

    R32 = _compose_rotation(coeffs).astype(np.float32)
    W = np.zeros((128, 128), dtype=np.float32)
    W[:DIM, :DIM] = R32
    W[DIM:, DIM:] = R32
    b2 = np.ascontiguousarray(np.tile(bias, 2).reshape(128, 1))

    key = (HALF, TILE_COLS, NCORES, 1)
    if key not in _BASS_CACHE:
        _BASS_CACHE[key] = _build_bass(
            half=HALF, tile_cols=TILE_COLS, n_cores=NCORES, reps=1,
            mm_dtype="f32", mode="preload", ring_split=True,
        )
    nc = _BASS_CACHE[key]

    in_maps = []
    for r in range(NCORES):
        xs = x[r * SHARD:(r + 1) * SHARD]
        in_maps.append(
            {"x2": _pack_shard(xs, TILE_COLS), "w": W, "b2": b2}
        )

    res = run_bass_kernel_spmd(
        nc, in_maps, core_ids=list(range(NCORES)), trace=False
    )

    out = np.empty((NROWS, DIM), dtype=np.float32)
    for r in range(NCORES):
        out[r * SHARD:(r + 1) * SHARD] = _unpack_shard(
            res.results[r]["o2"], TILE_COLS
        )
    return out
